# revision 2
# baseline (speedup 1.0000x reference)
"""GeometricModalityFusion kernel for 8 Trainium2 NeuronCores.

Strategy (per sharding_hint): data-parallel over batch B=32 across the
8 cores (4 batch elements per core); small weights replicated. Every
branch (projections, modality-axis softmax attention, angular/Cayley
scores) is independent per batch element, so no collectives are needed
beyond the final gather of per-core outputs.

The neuron backend cannot lower `acos`, so the kernel runs in two
device stages: stage 1 produces feats/mha plus the tiny (Bl,3,3) cosine
matrix and (Bl,3) Cayley volumes; the 3x3 arccos + softmax weights are
computed on host (microseconds); stage 2 applies the weighted sums and
output projection. feats stays resident on device between stages.

Self-contained: shapes hardcoded; takes FULL inputs, returns FULL output.
"""
import numpy as np
import jax
import jax.numpy as jnp

B, S, D, H = 32, 1024, 512, 8
HD = D // H
M = 3
DIMS = [768, 1024, 512]
LENS = [512, 256, 1024]
N_CORES = 8
BPC = B // N_CORES  # batch elements per core

_WEIGHT_KEYS = ['W0', 'b0', 'W1', 'b1', 'W2', 'b2',
                'Wq', 'bq', 'Wk', 'bk', 'Wv', 'bv', 'Wo', 'bo',
                'temperature', 'attention_weights', 'role_weights']


def _stage1(text, image, audio, w):
    t_abs = jnp.abs(w['temperature'])

    def proj_pad(x, W, b):
        p = x @ W + b
        pad = S - p.shape[1]
        return jnp.pad(p, ((0, 0), (0, pad), (0, 0)))

    feats = jnp.stack([proj_pad(text, w['W0'], w['b0']),
                       proj_pad(image, w['W1'], w['b1']),
                       proj_pad(audio, w['W2'], w['b2'])], axis=0)  # (M,Bl,S,D)
    Bl = feats.shape[1]

    # --- standard attention (softmax over the modality axis) ---
    q = (feats[0] @ w['Wq'] + w['bq']).reshape(Bl, H, S, HD)
    k = (feats @ w['Wk'] + w['bk'][None, None, None]).reshape(M, Bl, H, S, HD)
    v = (feats @ w['Wv'] + w['bv'][None, None, None]).reshape(M, Bl, H, S, HD)
    k = jnp.transpose(k, (1, 2, 0, 3, 4))       # (Bl,H,M,S,hd)
    v = jnp.transpose(v, (1, 2, 0, 3, 4))
    scores = jnp.einsum('bhsd,bhmsd->bhms', q, k) / (np.sqrt(HD) * t_abs)
    attn = jax.nn.softmax(scores, axis=2)
    mha = jnp.einsum('bhms,bhmsd->bhsd', attn, v)
    mha = jnp.transpose(mha, (0, 2, 1, 3)).reshape(Bl, S, D)

    # --- angular attention: only the tiny cosine matrix on device ---
    fn = feats / jnp.maximum(jnp.linalg.norm(feats, axis=-1, keepdims=True), 1e-12)
    cos = jnp.einsum('ibsd,jbsd->bij', fn, fn) / S   # (Bl,M,M)

    # --- Cayley-Menger volumes ---
    P = 1 + min(4, M - 1)
    vols = []
    for i in range(M):
        pts = [feats[i]]
        for j in range(min(4, M - 1)):
            ang = (j + 1) * np.pi / 4.0
            other = (i + j + 1) % M
            pts.append(feats[i] * np.cos(ang) + feats[other] * np.sin(ang))
        G = jnp.stack(pts, axis=1).reshape(Bl, P, S * D)
        gram = jnp.einsum('bpk,bqk->bpq', G, G)
        sq = jnp.einsum('bpk,bpk->bp', G, G)
        distsq = sq[:, :, None] + sq[:, None, :] - 2.0 * gram
        vols.append(distsq.mean(axis=(1, 2)))
    volumes = jnp.stack(vols, axis=1)                # (Bl,M)

    return feats, mha, cos, volumes


def _stage2(feats, mha, aw, cw, w):
    angular_out = jnp.einsum('bm,mbsd->bsd', aw, feats)
    cayley_out = jnp.einsum('bm,mbsd->bsd', cw, feats)
    ww = jax.nn.softmax(w['attention_weights'], axis=0)
    fused = ww[0] * mha + ww[1] * angular_out + ww[2] * cayley_out
    return fused @ w['Wo'] + w['bo']


_p1 = None
_p2 = None


def _get_pmapped():
    global _p1, _p2
    if _p1 is None:
        _p1 = jax.pmap(_stage1, in_axes=(0, 0, 0, None), axis_name='x')
        _p2 = jax.pmap(_stage2, in_axes=(0, 0, 0, 0, None), axis_name='x')
    return _p1, _p2


def kernel(**inputs) -> np.ndarray:
    text = np.asarray(inputs['text'], dtype=np.float32).reshape(N_CORES, BPC, LENS[0], DIMS[0])
    image = np.asarray(inputs['image'], dtype=np.float32).reshape(N_CORES, BPC, LENS[1], DIMS[1])
    audio = np.asarray(inputs['audio'], dtype=np.float32).reshape(N_CORES, BPC, LENS[2], DIMS[2])
    w = {k: jnp.asarray(inputs[k], dtype=np.float32) for k in _WEIGHT_KEYS}
    p1, p2 = _get_pmapped()

    feats, mha, cos, volumes = p1(text, image, audio, w)

    # host-side tiny math (arccos is not lowerable on neuron)
    t_abs = abs(float(np.asarray(inputs['temperature'])))
    role = np.asarray(inputs['role_weights'], dtype=np.float64)
    cos_h = np.asarray(cos, dtype=np.float64)                    # (8,BPC,3,3)
    angle = np.arccos(np.clip(cos_h, -1 + 1e-7, 1 - 1e-7))
    contrib = role[None, None, None, :] * np.exp(-angle / t_abs)
    offdiag = 1.0 - np.eye(M)
    ang_scores = (contrib * offdiag[None, None]).sum(axis=-1)    # (8,BPC,3)
    e = np.exp(ang_scores - ang_scores.max(axis=-1, keepdims=True))
    aw = (e / e.sum(axis=-1, keepdims=True)).astype(np.float32)

    vol_h = np.asarray(volumes, dtype=np.float64) / t_abs        # (8,BPC,3)
    e2 = np.exp(vol_h - vol_h.max(axis=-1, keepdims=True))
    cw = (e2 / e2.sum(axis=-1, keepdims=True)).astype(np.float32)

    out = p2(feats, mha, jnp.asarray(aw), jnp.asarray(cw), w)    # (8,BPC,S,D)
    return np.asarray(out).reshape(B, S, D).astype(np.float32)
